# revision 1
# baseline (speedup 1.0000x reference)
"""Trainium2 Bass kernel for nn_Block_7696581394709 (dense transformer block).

Sharding: 8 cores = 4 batches x 2 head-groups (8 heads each).
Per core: LN1 -> fused in_proj (qT,kT transposed; v,p,smear,dpos natural) ->
token-shift smear on kT -> causal attention with relpos bias folded into the
exp bias (per-i-tile clamped offset keeps exp in fp32 range; softmax
denominator comes free via a ones-column appended to v) -> silu(p) gate ->
out_proj partial -> pair ReduceScatter -> LN2 -> output token-half.

Matmuls run as float32r (full PE rate at free-dim>=256, ~tf32 precision) for
in_proj / QK / out_proj, bf16 for the attention AV matmul.
"""
import math
import os
import sys

sys.path.insert(0, "/opt/trn_rl_repo")

import numpy as np

import bass_rust
import concourse.bass as bass
import concourse.mybir as mybir
from concourse.tile import TileContext
from concourse.masks import make_identity, make_lower_triangular
from concourse.bass_utils import run_bass_kernel_spmd

F32 = mybir.dt.float32
F32R = mybir.dt.float32r
BF16 = mybir.dt.bfloat16
ALU = mybir.AluOpType
ACTF = mybir.ActivationFunctionType
AX = mybir.AxisListType

N_CORES = 8
PAIRS = [[0, 1], [2, 3], [4, 5], [6, 7]]

B, T, D = 4, 1024, 1024
H, HG, DH = 16, 8, 128
E = 2048
EG = HG * DH  # 1024 cols per group for each of q/k/v/p
NT = T // 128  # 8 token tiles
ND = D // 128  # 8 d tiles
EPS = 1e-5
CLIP = 70.0
NEGM = -1e9
RSQ_DH = 1.0 / math.sqrt(DH)


def _r(ap):
    return ap.bitcast(F32R)


def _legalize_waits(nc):
    """This walrus build accepts at most 1 embedded sem-wait per normal
    instruction (2 on EventSemaphore). Hoist excess waits onto EventSemaphore
    instructions inserted before the offending instruction (same engine)."""
    for f in nc.m.functions:
        for bb in f.blocks:
            out = []
            changed = False
            for inst in bb.instructions:
                si = inst.sync_info
                waits = list(si.on_wait) if si is not None else []
                cap = 2 if isinstance(inst, mybir.InstEventSemaphore) else 1
                if len(waits) > cap:
                    extra, keep = waits[:-cap], waits[-cap:]
                    for i in range(0, len(extra), 2):
                        ev = mybir.InstEventSemaphore(
                            name=nc.get_next_instruction_name(), ins=[], outs=[]
                        )
                        ev.engine = inst.engine
                        ev.sync_info = bass_rust.SyncInfo(
                            on_wait=extra[i : i + 2], on_update=[]
                        )
                        nc.register_instruction(ev, overwrite=True)
                        out.append(ev)
                    si.on_wait = keep
                    inst.sync_info = si
                    changed = True
                out.append(inst)
            if changed:
                bb.instructions = out
    return nc


def build_program():
    WDT = BF16 if os.environ.get("KBF16", "0") == "1" else F32R
    nc = bass.Bass(num_devices=N_CORES)

    x_in = nc.declare_dram_parameter("x", [T, D], F32, False)
    wqk_in = nc.declare_dram_parameter("wqk", [D, 2 * EG], WDT, False)
    wvp_in = nc.declare_dram_parameter("wvp", [D, 2 * EG + 16], WDT, False)
    wout_in = nc.declare_dram_parameter("wout", [EG, D], WDT, False)
    bqk_in = nc.declare_dram_parameter("bqk", [128, 16], F32, False)
    bvp_in = nc.declare_dram_parameter("bvp", [1, 2 * EG + 16], F32, False)
    ln1w_in = nc.declare_dram_parameter("ln1w", [128, ND], F32, False)
    ln1b_in = nc.declare_dram_parameter("ln1b", [128, ND], F32, False)
    ln2w_in = nc.declare_dram_parameter("ln2w", [1, D], F32, False)
    ln2b_in = nc.declare_dram_parameter("ln2b", [1, D], F32, False)
    fac_in = nc.declare_dram_parameter("fac", [1, HG], F32, False)
    out_ext = nc.declare_dram_parameter("out", [T // 2, D], F32, True)
    KDBG = os.environ.get("KDBG", "0") == "1"
    dbg = {}
    if KDBG:
        dbg["hT0"] = nc.declare_dram_parameter("dbg_hT0", [128, T], F32, True)
        dbg["qT0"] = nc.declare_dram_parameter("dbg_qT0", [128, T], F32, True)
        dbg["keff0"] = nc.declare_dram_parameter("dbg_keff0", [128, T], F32, True)
        dbg["posT"] = nc.declare_dram_parameter("dbg_posT", [8, T], F32, True)
        dbg["sbc0"] = nc.declare_dram_parameter("dbg_sbc0", [128, T], F32, True)
        dbg["posrel00"] = nc.declare_dram_parameter("dbg_posrel00", [128, NT], F32, True)
        dbg["expS0"] = nc.declare_dram_parameter("dbg_expS0", [128, 8 * 512], BF16, True)
        dbg["gT0"] = nc.declare_dram_parameter("dbg_gT0", [128, T], F32, True)
        dbg["vaug0"] = nc.declare_dram_parameter("dbg_vaug0", [128, HG * (DH + 1)], BF16, True)
        dbg["silup0"] = nc.declare_dram_parameter("dbg_silup0", [128, EG], BF16, True)
        dbg["partial"] = nc.declare_dram_parameter("dbg_partial", [T, D], F32, True)

    with TileContext(nc) as tc:
        import contextlib

        es = contextlib.ExitStack()
        with es:
            const = es.enter_context(tc.tile_pool(name="const", bufs=1))
            dram = es.enter_context(tc.tile_pool(name="dram", bufs=1, space="DRAM"))


            rs_in_a = dram.tile([T // 2, D], F32, tag="rs_in_a")
            rs_in_b = dram.tile([T // 2, D], F32, tag="rs_in_b")
            rs_out_a = dram.tile([T // 4, D], F32, tag="rs_out_a")
            rs_out_b = dram.tile([T // 4, D], F32, tag="rs_out_b")

            # ---- constants ----
            pbc_es = contextlib.ExitStack()
            pbc = pbc_es.enter_context(tc.tile_pool(name="pbc", bufs=1, space="PSUM"))
            ones1 = const.tile([1, 128], F32, tag="ones1")
            nc.vector.memset(ones1[:], 1.0)
            ident = const.tile([128, 128], F32, tag="ident")
            make_identity(nc, ident[:])
            mtri = const.tile([128, 128], F32, tag="mtri")
            make_lower_triangular(nc, mtri[:], val=NEGM, diag=False)

            bqk_t = const.tile([128, 16], F32, tag="bqk_t")
            nc.sync.dma_start(out=bqk_t[:], in_=bqk_in[:])
            ln1w_t = const.tile([128, ND], F32, tag="ln1w_t")
            nc.sync.dma_start(out=ln1w_t[:], in_=ln1w_in[:])
            ln1b_t = const.tile([128, ND], F32, tag="ln1b_t")
            nc.sync.dma_start(out=ln1b_t[:], in_=ln1b_in[:])

            bvp_row = const.tile([1, 2 * EG + 16], F32, tag="bvp_row")
            nc.sync.dma_start(out=bvp_row[:], in_=bvp_in[:])
            fac_row = const.tile([1, HG], F32, tag="fac_row")
            nc.sync.dma_start(out=fac_row[:], in_=fac_in[:])

            # broadcast rows down 128 partitions via K=1 matmuls
            bvp_b = const.tile([128, 2 * EG + 16], F32, tag="bvp_b")
            for nch in range(5):
                c0 = nch * 512
                w = min(512, 2 * EG + 16 - c0)
                pb = pbc.tile([128, 512], F32, tag="pb")
                nc.tensor.matmul(
                    pb[:, :w], ones1[:], bvp_row[:, c0 : c0 + w],
                    start=True, stop=True,
                )
                nc.scalar.copy(bvp_b[:, c0 : c0 + w], pb[:, :w])
            fac_b = const.tile([128, HG], F32, tag="fac_b")
            pb = pbc.tile([128, 512], F32, tag="pb")
            nc.tensor.matmul(pb[:, :HG], ones1[:], fac_row[:], start=True, stop=True)
            nc.scalar.copy(fac_b[:], pb[:, :HG])
            pbc_es.close()

            REPS = int(os.environ.get("KREPS", "1"))
            for _rep in range(REPS):
                # ================= PHASE A: LN1 + transpose =================
                hT = []
                with tc.tile_pool(name="pA", bufs=2) as pa, tc.tile_pool(
                    name="pAp", bufs=2, space="PSUM"
                ) as pap:
                    xn = []
                    for it in range(NT):
                        x_t = pa.tile([128, D], F32, tag="x_t", bufs=8)
                        nc.sync.dma_start(out=x_t[:], in_=x_in[it * 128 : (it + 1) * 128, :])
                        rsum = pa.tile([128, 1], F32, tag="rsum", bufs=3)
                        nc.vector.reduce_sum(rsum[:], x_t[:], axis=AX.X)
                        sqt = pa.tile([128, D], F32, tag="sqt", bufs=2)
                        sqsum = pa.tile([128, 1], F32, tag="sqsum", bufs=3)
                        nc.scalar.activation(
                            sqt[:], x_t[:], ACTF.Square, accum_out=sqsum[:]
                        )
                        mu = pa.tile([128, 1], F32, tag="mu", bufs=3)
                        nc.vector.tensor_scalar(
                            out=mu[:], in0=rsum[:], scalar1=1.0 / D, scalar2=None,
                            op0=ALU.mult,
                        )
                        var = pa.tile([128, 1], F32, tag="var", bufs=3)
                        nc.vector.tensor_scalar(
                            out=var[:], in0=sqsum[:], scalar1=1.0 / D, scalar2=EPS,
                            op0=ALU.mult, op1=ALU.add,
                        )
                        mu2 = pa.tile([128, 1], F32, tag="mu2", bufs=3)
                        nc.vector.tensor_mul(mu2[:], mu[:], mu[:])
                        nc.vector.tensor_sub(var[:], var[:], mu2[:])
                        sd_ = pa.tile([128, 1], F32, tag="sd_", bufs=3)
                        nc.scalar.sqrt(sd_[:], var[:])
                        rs = pa.tile([128, 1], F32, tag="rs", bufs=3)
                        nc.vector.reciprocal(rs[:], sd_[:])
                        nmrs = pa.tile([128, 1], F32, tag="nmrs", bufs=3)
                        nc.vector.tensor_scalar(
                            out=nmrs[:], in0=mu[:], scalar1=rs[:], scalar2=-1.0,
                            op0=ALU.mult, op1=ALU.mult,
                        )
                        xn_t = pa.tile([128, D], F32, tag="xn_t", bufs=8)
                        nc.vector.tensor_scalar(
                            out=xn_t[:], in0=x_t[:], scalar1=rs[:], scalar2=nmrs[:],
                            op0=ALU.mult, op1=ALU.add,
                        )
                        xn.append(xn_t)

                    for dt in range(ND):
                        hT_t = const.tile([128, T], WDT, tag="hT", bufs=ND, name=f"hT{dt}")
                        hT.append(hT_t)
                        for ith in range(2):
                            pt = pap.tile([128, 512], F32, tag="pt")
                            for q in range(4):
                                it = ith * 4 + q
                                nc.tensor.transpose(
                                    pt[:, q * 128 : (q + 1) * 128],
                                    xn[it][:, dt * 128 : (dt + 1) * 128],
                                    ident[:],
                                )
                            nc.scalar.activation(
                                hT_t[:, ith * 512 : (ith + 1) * 512], pt[:],
                                ACTF.Identity,
                                bias=ln1b_t[:, dt : dt + 1],
                                scale=ln1w_t[:, dt : dt + 1],
                            )

                if KDBG:
                    nc.sync.dma_start(out=dbg["hT0"][:], in_=hT[0][:].bitcast(F32))
                # ============ PHASE B: natural in_proj (v, p, sd) ============
                v_aug, silup, sd_nat = [], [], []
                gT = []
                for h in range(HG):
                    gT.append(const.tile([128, T], WDT, tag="gT", bufs=HG, name=f"gT{h}"))
                for it in range(NT):
                    v_aug.append(const.tile([128, HG * (DH + 1)], BF16, tag="v_aug", bufs=NT, name=f"v_aug{it}"))
                    silup.append(const.tile([128, EG], BF16, tag="silup", bufs=NT, name=f"silup{it}"))
                    sd_nat.append(const.tile([128, 16], F32, tag="sd_nat", bufs=NT, name=f"sd_nat{it}"))

                # sd chunk first: the pos/smear prep chain it feeds gates all
                # attention heads, so let it overlap the big v/p chunks.
                NCH = [("sd", 2048, 16), ("v", 0, 512), ("v", 512, 512),
                       ("p", 1024, 512), ("p", 1536, 512)]
                with tc.tile_pool(name="pB", bufs=2) as pb_, tc.tile_pool(
                    name="pBp", bufs=2, space="PSUM"
                ) as pbp:
                    for it in range(NT):
                        nc.vector.memset(v_aug[it][:], 1.0)
                    for kind, c0, w in NCH:
                        wvp_t = pb_.tile([128, ND * 512], WDT, tag="wvp_t", bufs=2)
                        nc.sync.dma_start(
                            out=wvp_t[:, : ND * w].rearrange("p (kt c) -> p kt c", c=w),
                            in_=wvp_in[:, c0 : c0 + w].rearrange(
                                "(kt p) c -> p kt c", p=128
                            ),
                        )
                        for it in range(NT):
                            ps = pbp.tile([128, 512], F32, tag="ps")
                            for kt in range(ND):
                                nc.tensor.matmul(
                                    ps[:, :w],
                                    hT[kt][:, it * 128 : (it + 1) * 128],
                                    wvp_t[:, kt * w : (kt + 1) * w],
                                    start=(kt == 0),
                                    stop=(kt == ND - 1),
                                )
                            if kind == "v":  # v columns -> v_aug (bf16, +bias)
                                h0 = c0 // 128
                                nc.vector.tensor_tensor(
                                    out=v_aug[it]
                                    .rearrange("p (h c) -> p h c", c=DH + 1)[
                                        :, h0 : h0 + 4, 0:DH
                                    ],
                                    in0=ps[:, :w].rearrange("p (h c) -> p h c", c=DH),
                                    in1=bvp_b[:, c0 : c0 + w].rearrange(
                                        "p (h c) -> p h c", c=DH
                                    ),
                                    op=ALU.add,
                                )
                            elif kind == "p":  # p columns -> silu(p) (bf16)
                                pt_ = pb_.tile([128, 512], F32, tag="pt_", bufs=3)
                                nc.vector.tensor_tensor(
                                    out=pt_[:], in0=ps[:, :w], in1=bvp_b[:, c0 : c0 + w],
                                    op=ALU.add,
                                )
                                ps0 = c0 - 1024
                                nc.scalar.activation(
                                    silup[it][:, ps0 : ps0 + 512],
                                    pt_[:], ACTF.Silu,
                                )
                            else:  # smear/dpos columns
                                nc.vector.tensor_tensor(
                                    out=sd_nat[it][:], in0=ps[:, :w],
                                    in1=bvp_b[:, c0 : c0 + w], op=ALU.add,
                                )

                # ---- pos / smear prep ----
                # All partition bases must be 32-aligned on this toolchain, so
                # per-head rows are produced at partition 0 via PE transposes of
                # base-0 columns.
                posrel = [[None] * NT for _ in range(HG)]
                sigsm_nat = []
                for it in range(NT):
                    ssn = const.tile(
                        [128, 8], F32, tag="sigsm_nat", bufs=NT, name=f"sigsm{it}"
                    )
                    nc.scalar.activation(ssn[:], sd_nat[it][:, 0:8], ACTF.Sigmoid)
                    sigsm_nat.append(ssn)
                with tc.tile_pool(name="pP", bufs=1) as pp, tc.tile_pool(
                    name="pPp", bufs=2, space="PSUM"
                ) as ppp:
                    dpT = pp.tile([8, T], F32, tag="dpT")
                    for ith in range(2):
                        pt = ppp.tile([128, 512], F32, tag="pt")
                        for q in range(4):
                            it = ith * 4 + q
                            nc.tensor.transpose(
                                pt[0:8, q * 128 : (q + 1) * 128],
                                sd_nat[it][:, 8:16], ident[:],
                            )
                        nc.scalar.copy(dpT[:, ith * 512 : (ith + 1) * 512], pt[0:8, :])
                    sigdp = pp.tile([8, T], F32, tag="sigdp")
                    nc.scalar.activation(sigdp[:], dpT[:], ACTF.Sigmoid)
                    zer = pp.tile([8, T], F32, tag="zer")
                    nc.vector.memset(zer[:], 0.0)
                    posT = pp.tile([8, T], F32, tag="posT")
                    nc.vector.tensor_tensor_scan(
                        posT[:], sigdp[:], zer[:], 0.0, op0=ALU.add, op1=ALU.add
                    )
                    # c[h, t] = min(pos[last of tile t], pos[first of tile t] + CLIP)
                    if KDBG:
                        nc.sync.dma_start(out=dbg["posT"][:], in_=posT[:])
                    cT = pp.tile([8, NT], F32, tag="cT")
                    nc.vector.tensor_scalar(
                        out=cT[:], in0=posT[:, 0 : T : 128], scalar1=CLIP, scalar2=None,
                        op0=ALU.add,
                    )
                    nc.vector.tensor_tensor(
                        out=cT[:], in0=cT[:], in1=posT[:, 127 : T : 128], op=ALU.min
                    )
                    cT2 = pp.tile([NT, 8], F32, tag="cT2")
                    pt = ppp.tile([128, 512], F32, tag="pt")
                    nc.tensor.transpose(pt[0:NT, 0:8], cT[:], ident[0:8, 0:8])
                    nc.scalar.copy(cT2[:], pt[0:NT, 0:8])
                    # pos in natural layout [token, head]
                    pos_nat = []
                    for jt in range(NT):
                        pn = pp.tile([128, 8], F32, tag="pos_nat", bufs=NT, name=f"pos_nat{jt}")
                        pt = ppp.tile([128, 512], F32, tag="pt")
                        nc.tensor.transpose(
                            pt[:, 0:8], posT[:, jt * 128 : (jt + 1) * 128],
                            ident[0:8, 0:8],
                        )
                        nc.scalar.copy(pn[:], pt[:, 0:8])
                        pos_nat.append(pn)
                    for h in range(HG):
                        pt = ppp.tile([128, 512], F32, tag="pt")
                        nc.tensor.transpose(
                            pt[0:1, 0:NT], cT2[:, h : h + 1], ident[0:8, 0:8]
                        )
                        crow = pp.tile([1, NT], F32, tag="crow", bufs=2)
                        nc.scalar.copy(crow[:], pt[0:1, 0:NT])
                        cb = pp.tile([128, NT], F32, tag="cb", bufs=2)
                        pt2 = ppp.tile([128, 512], F32, tag="pt")
                        nc.tensor.matmul(
                            pt2[:, :NT], ones1[:], crow[:], start=True, stop=True
                        )
                        nc.scalar.copy(cb[:], pt2[:, :NT])
                        for jt in range(NT):
                            pr = const.tile([128, NT], F32, tag="posrel", bufs=HG * NT, name=f"posrel{h}_{jt}")
                            # pos_j - c_t  ==  (c_t - pos_j) * -1
                            nc.vector.tensor_scalar(
                                out=pr[:], in0=cb[:], scalar1=pos_nat[jt][:, h : h + 1],
                                scalar2=-1.0, op0=ALU.subtract, op1=ALU.mult,
                            )
                            posrel[h][jt] = pr
                            if KDBG and h == 0 and jt == 0:
                                nc.sync.dma_start(out=dbg["posrel00"][:], in_=pr[:])

                # ================= PHASE C: per-head attention =================
                with tc.tile_pool(name="pC", bufs=2) as pc, tc.tile_pool(
                    name="pCq", bufs=2, space="PSUM"
                ) as pcq, tc.tile_pool(
                    name="pCs", bufs=2, space="PSUM"
                ) as pcs, tc.tile_pool(
                    name="pCo", bufs=2, space="PSUM"
                ) as pco, tc.tile_pool(
                    name="pCt", bufs=2, space="PSUM"
                ) as pct:
                    for h in range(HG):
                        # -- in_proj q,k (transposed outputs) --
                        wq_t = pc.tile([128, ND * 128], WDT, tag="wq_t", bufs=2)
                        nc.sync.dma_start(
                            out=wq_t[:].rearrange("p (kt c) -> p kt c", c=128),
                            in_=wqk_in[:, h * 128 : (h + 1) * 128].rearrange(
                                "(kt p) c -> p kt c", p=128
                            ),
                        )
                        wk_t = pc.tile([128, ND * 128], WDT, tag="wk_t", bufs=2)
                        nc.sync.dma_start(
                            out=wk_t[:].rearrange("p (kt c) -> p kt c", c=128),
                            in_=wqk_in[:, EG + h * 128 : EG + (h + 1) * 128].rearrange(
                                "(kt p) c -> p kt c", p=128
                            ),
                        )
                        qT = pc.tile([128, T], F32R, tag="qT", bufs=3)
                        kT = pc.tile([128, T], F32, tag="kT", bufs=2)
                        for part, (wt, dst, ct) in enumerate(
                            [(wq_t, qT, h), (wk_t, kT, HG + h)]
                        ):
                            for ic in range(2):
                                ps = pcq.tile([128, 512], F32, tag="ps")
                                for kt in range(ND):
                                    nc.tensor.matmul(
                                        ps[:],
                                        wt[:, kt * 128 : (kt + 1) * 128],
                                        hT[kt][:, ic * 512 : (ic + 1) * 512],
                                        start=(kt == 0),
                                        stop=(kt == ND - 1),
                                    )
                                nc.scalar.activation(
                                    dst[:, ic * 512 : (ic + 1) * 512], ps[:],
                                    ACTF.Identity, bias=bqk_t[:, ct : ct + 1], scale=1.0,
                                )
                        # -- token-shift smear on kT (free-dim shift) --
                        sbc = pc.tile([128, T], F32, tag="sbc", bufs=2)
                        smrow = pc.tile([1, T], F32, tag="smrow", bufs=2)
                        for ith in range(2):
                            ps = pcq.tile([128, 512], F32, tag="ps")
                            for q in range(4):
                                it = ith * 4 + q
                                nc.tensor.transpose(
                                    ps[0:1, q * 128 : (q + 1) * 128],
                                    sigsm_nat[it][:, h : h + 1], ident[:],
                                )
                            nc.scalar.copy(
                                smrow[:, ith * 512 : (ith + 1) * 512], ps[0:1, :]
                            )
                        for ic in range(2):
                            ps = pcq.tile([128, 512], F32, tag="ps")
                            nc.tensor.matmul(
                                ps[:], ones1[:],
                                smrow[:, ic * 512 : (ic + 1) * 512],
                                start=True, stop=True,
                            )
                            nc.scalar.copy(sbc[:, ic * 512 : (ic + 1) * 512], ps[:])
                        keff = pc.tile([128, T], F32R, tag="keff", bufs=2)
                        kd = pc.tile([128, T], F32, tag="kd", bufs=2)
                        nc.vector.tensor_sub(kd[:, 1:T], kT[:, 0 : T - 1], kT[:, 1:T])
                        nc.vector.tensor_mul(kd[:, 1:T], kd[:, 1:T], sbc[:, 1:T])
                        nc.vector.tensor_add(keff[:, 1:T], kd[:, 1:T], kT[:, 1:T])
                        # keff[:,0] = k0 - s0*k0
                        nc.vector.tensor_mul(kd[:, 0:1], kT[:, 0:1], sbc[:, 0:1])
                        nc.vector.tensor_sub(keff[:, 0:1], kT[:, 0:1], kd[:, 0:1])
                        if KDBG and h == 0:
                            nc.sync.dma_start(out=dbg["qT0"][:], in_=qT[:].bitcast(F32))
                            nc.sync.dma_start(out=dbg["keff0"][:], in_=keff[:].bitcast(F32))
                            nc.sync.dma_start(out=dbg["sbc0"][:], in_=sbc[:])

                        # -- scores + softmax + AV --
                        for ic in range(2):
                            njt = 4 if ic == 0 else 8
                            expS = []
                            for jt in range(njt):
                                ps = pcs.tile([128, 512], F32, tag="s_ps")
                                # columns i < jt*128 are above the causal
                                # diagonal and never read by the exp stage;
                                # trim them, but keep N >= 256 (f32r runs at
                                # 1/4 rate below that, erasing the win).
                                lo = max(0, jt * 128 - ic * 512)
                                if lo > 256:
                                    lo = 0
                                nc.tensor.matmul(
                                    ps[:, lo:512],
                                    keff[:, jt * 128 : (jt + 1) * 128],
                                    qT[:, ic * 512 + lo : (ic + 1) * 512],
                                    start=True, stop=True,
                                )
                                ex = pc.tile([128, 512], BF16, tag="expS", bufs=12)
                                expS.append(ex)
                                for isub in range(4):
                                    it = ic * 4 + isub
                                    if jt > it:
                                        continue
                                    if jt == it:
                                        dtmp = pc.tile(
                                            [128, 128], F32, tag="dtmp", bufs=3
                                        )
                                        nc.vector.tensor_add(
                                            dtmp[:],
                                            ps[:, isub * 128 : (isub + 1) * 128],
                                            mtri[:],
                                        )
                                        src = dtmp[:]
                                    else:
                                        src = ps[:, isub * 128 : (isub + 1) * 128]
                                    nc.scalar.activation(
                                        ex[:, isub * 128 : (isub + 1) * 128], src,
                                        ACTF.Exp,
                                        bias=posrel[h][jt][:, it : it + 1],
                                        scale=fac_b[:, h : h + 1],
                                    )
                            if KDBG and h == 0 and ic == 1:
                                for jt in range(njt):
                                    nc.sync.dma_start(
                                        out=dbg["expS0"][:, jt * 512 : (jt + 1) * 512],
                                        in_=expS[jt][:],
                                    )
                            ptg = pct.tile([128, 512], F32, tag="ptg")
                            for isub in range(4):
                                it = ic * 4 + isub
                                po = pco.tile([128, DH + 1], F32, tag="po")
                                for jt in range(it + 1):
                                    nc.tensor.matmul(
                                        po[:],
                                        expS[jt][:, isub * 128 : (isub + 1) * 128],
                                        v_aug[jt][
                                            :, h * (DH + 1) : (h + 1) * (DH + 1)
                                        ],
                                        start=(jt == 0),
                                        stop=(jt == it),
                                    )
                                rcp = pc.tile([128, 1], F32, tag="rcp", bufs=4)
                                nc.vector.reciprocal(rcp[:], po[:, DH : DH + 1])
                                gb = pc.tile([128, 128], F32, tag="gb", bufs=6)
                                nc.vector.scalar_tensor_tensor(
                                    out=gb[:], in0=po[:, 0:DH], scalar=rcp[:],
                                    in1=silup[it][:, h * 128 : (h + 1) * 128],
                                    op0=ALU.mult, op1=ALU.mult,
                                )
                                # transpose g block into gT[h]
                                nc.tensor.transpose(
                                    ptg[:, (it % 4) * 128 : (it % 4 + 1) * 128], gb[:],
                                    ident[:],
                                )
                                if it % 4 == 3:
                                    nc.scalar.copy(
                                        gT[h][:, ic * 512 : (ic + 1) * 512], ptg[:]
                                    )

                if KDBG:
                    nc.sync.dma_start(out=dbg["gT0"][:], in_=gT[0][:].bitcast(F32))
                    nc.sync.dma_start(out=dbg["vaug0"][:], in_=v_aug[0][:])
                    nc.sync.dma_start(out=dbg["silup0"][:], in_=silup[0][:])
                # ================= PHASE D: out_proj =================
                with tc.tile_pool(name="pD", bufs=2) as pd_, tc.tile_pool(
                    name="pDp", bufs=2, space="PSUM"
                ) as pdp:
                    wout_t = []
                    for et in range(ND):
                        wt = pd_.tile([128, D], WDT, tag="wout_t", bufs=ND, name=f"wout{et}")
                        nc.sync.dma_start(
                            out=wt[:], in_=wout_in[et * 128 : (et + 1) * 128, :]
                        )
                        wout_t.append(wt)
                    for it in range(NT):
                        for nch in range(2):
                            ps = pdp.tile([128, 512], F32, tag="ps")
                            for et in range(ND):
                                nc.tensor.matmul(
                                    ps[:],
                                    gT[et][:, it * 128 : (it + 1) * 128],
                                    wout_t[et][:, nch * 512 : (nch + 1) * 512],
                                    start=(et == 0),
                                    stop=(et == ND - 1),
                                )
                            ot = pd_.tile([128, 512], F32, tag="ot", bufs=3)
                            nc.scalar.copy(ot[:], ps[:])
                            rs_dst = rs_in_a if it < 4 else rs_in_b
                            nc.sync.dma_start(
                                out=rs_dst[
                                    (it % 4) * 128 : (it % 4 + 1) * 128,
                                    nch * 512 : (nch + 1) * 512,
                                ],
                                in_=ot[:],
                            )
                            if KDBG:
                                nc.sync.dma_start(
                                    out=dbg["partial"][
                                        it * 128 : (it + 1) * 128,
                                        nch * 512 : (nch + 1) * 512,
                                    ],
                                    in_=ot[:],
                                )

                # ============ PHASE E: pair ReduceScatter + LN2 ============
                nc.gpsimd.collective_compute(
                    "ReduceScatter", ALU.add, replica_groups=PAIRS,
                    ins=[rs_in_a[:]], outs=[rs_out_a[:]],
                )
                nc.gpsimd.collective_compute(
                    "ReduceScatter", ALU.add, replica_groups=PAIRS,
                    ins=[rs_in_b[:]], outs=[rs_out_b[:]],
                )
                with tc.tile_pool(name="pE", bufs=1) as pe, tc.tile_pool(
                    name="pEp", bufs=2, space="PSUM"
                ) as pep:
                    ln2w_row = pe.tile([1, D], F32, tag="ln2w_row")
                    nc.sync.dma_start(out=ln2w_row[:], in_=ln2w_in[:])
                    ln2b_row = pe.tile([1, D], F32, tag="ln2b_row")
                    nc.sync.dma_start(out=ln2b_row[:], in_=ln2b_in[:])
                    ln2w_b = pe.tile([128, D], F32, tag="ln2w_b")
                    ln2b_b = pe.tile([128, D], F32, tag="ln2b_b")
                    for dst, row in [(ln2w_b, ln2w_row), (ln2b_b, ln2b_row)]:
                        for nch in range(2):
                            ps = pep.tile([128, 512], F32, tag="ps")
                            nc.tensor.matmul(
                                ps[:], ones1[:],
                                row[:, nch * 512 : (nch + 1) * 512],
                                start=True, stop=True,
                            )
                            nc.scalar.copy(dst[:, nch * 512 : (nch + 1) * 512], ps[:])
                    for it in range(4):
                        y_t = pe.tile([128, D], F32, tag="y_t", bufs=3)
                        rs_src = rs_out_a if it < 2 else rs_out_b
                        nc.sync.dma_start(
                            out=y_t[:], in_=rs_src[(it % 2) * 128 : (it % 2 + 1) * 128, :]
                        )
                        rsum = pe.tile([128, 1], F32, tag="rsum", bufs=3)
                        nc.vector.reduce_sum(rsum[:], y_t[:], axis=AX.X)
                        sqt = pe.tile([128, D], F32, tag="sqt", bufs=2)
                        sqsum = pe.tile([128, 1], F32, tag="sqsum", bufs=3)
                        nc.scalar.activation(sqt[:], y_t[:], ACTF.Square, accum_out=sqsum[:])
                        mu = pe.tile([128, 1], F32, tag="mu", bufs=3)
                        nc.vector.tensor_scalar(
                            out=mu[:], in0=rsum[:], scalar1=1.0 / D, scalar2=None,
                            op0=ALU.mult,
                        )
                        var = pe.tile([128, 1], F32, tag="var", bufs=3)
                        nc.vector.tensor_scalar(
                            out=var[:], in0=sqsum[:], scalar1=1.0 / D, scalar2=EPS,
                            op0=ALU.mult, op1=ALU.add,
                        )
                        mu2 = pe.tile([128, 1], F32, tag="mu2", bufs=3)
                        nc.vector.tensor_mul(mu2[:], mu[:], mu[:])
                        nc.vector.tensor_sub(var[:], var[:], mu2[:])
                        sd_ = pe.tile([128, 1], F32, tag="sd_", bufs=3)
                        nc.scalar.sqrt(sd_[:], var[:])
                        rs = pe.tile([128, 1], F32, tag="rs", bufs=3)
                        nc.vector.reciprocal(rs[:], sd_[:])
                        nmrs = pe.tile([128, 1], F32, tag="nmrs", bufs=3)
                        nc.vector.tensor_scalar(
                            out=nmrs[:], in0=mu[:], scalar1=rs[:], scalar2=-1.0,
                            op0=ALU.mult, op1=ALU.mult,
                        )
                        yn = pe.tile([128, D], F32, tag="yn", bufs=3)
                        nc.vector.tensor_scalar(
                            out=yn[:], in0=y_t[:], scalar1=rs[:], scalar2=nmrs[:],
                            op0=ALU.mult, op1=ALU.add,
                        )
                        yf = pe.tile([128, D], F32, tag="yf", bufs=3)
                        nc.vector.tensor_mul(yf[:], yn[:], ln2w_b[:])
                        nc.vector.tensor_add(yf[:], yf[:], ln2b_b[:])
                        nc.sync.dma_start(
                            out=out_ext[it * 128 : (it + 1) * 128, :], in_=yf[:]
                        )

    _legalize_waits(nc)
    return nc


_PROGRAM = None


def _get_program():
    global _PROGRAM
    if _PROGRAM is None:
        _PROGRAM = build_program()
    return _PROGRAM


def make_in_maps(inputs):
    import ml_dtypes
    wcast = (
        (lambda a: np.ascontiguousarray(a).astype(ml_dtypes.bfloat16))
        if os.environ.get("KBF16", "0") == "1"
        else (lambda a: np.ascontiguousarray(a, dtype=np.float32))
    )
    x = np.ascontiguousarray(np.asarray(inputs["x"], dtype=np.float32))
    Wm = np.asarray(inputs["W_merged"], dtype=np.float32)
    bm = np.asarray(inputs["b_merged"], dtype=np.float32)
    ln1_g = np.asarray(inputs["ln1_g"], dtype=np.float32)
    ln1_b = np.asarray(inputs["ln1_b"], dtype=np.float32)
    log_scale = np.asarray(inputs["log_scale"], dtype=np.float32)
    W_out = np.asarray(inputs["W_out"], dtype=np.float32)
    ln2_g = np.asarray(inputs["ln2_g"], dtype=np.float32)
    ln2_b = np.asarray(inputs["ln2_b"], dtype=np.float32)

    fac_all = np.exp(-2.0 * log_scale) * RSQ_DH  # [H]

    in_maps = []
    for c in range(N_CORES):
        b, g = c // 2, c % 2
        cs = g * EG
        wq = Wm[:, cs : cs + EG]
        wk = Wm[:, E + cs : E + cs + EG]
        wv = Wm[:, 2 * E + cs : 2 * E + cs + EG]
        wp = Wm[:, 3 * E + cs : 3 * E + cs + EG]
        wsm = Wm[:, 4 * E + g * HG : 4 * E + (g + 1) * HG]
        wdp = Wm[:, 4 * E + H + g * HG : 4 * E + H + (g + 1) * HG]
        bq = bm[cs : cs + EG]
        bk = bm[E + cs : E + cs + EG]
        bv = bm[2 * E + cs : 2 * E + cs + EG]
        bp = bm[3 * E + cs : 3 * E + cs + EG]
        bsm = bm[4 * E + g * HG : 4 * E + (g + 1) * HG]
        bdp = bm[4 * E + H + g * HG : 4 * E + H + (g + 1) * HG]
        in_maps.append(
            {
                "x": x[b],
                "wqk": wcast(np.concatenate([wq, wk], axis=1)),
                "wvp": wcast(np.concatenate([wv, wp, wsm, wdp], axis=1)),
                "wout": wcast(W_out[cs : cs + EG, :]),
                "bqk": np.ascontiguousarray(
                    np.concatenate([bq, bk]).reshape(16, 128).T
                ),
                "bvp": np.ascontiguousarray(
                    np.concatenate([bv, bp, bsm, bdp])[None, :]
                ),
                "ln1w": np.ascontiguousarray(ln1_g.reshape(ND, 128).T),
                "ln1b": np.ascontiguousarray(ln1_b.reshape(ND, 128).T),
                "ln2w": np.ascontiguousarray(ln2_g[None, :]),
                "ln2b": np.ascontiguousarray(ln2_b[None, :]),
                "fac": np.ascontiguousarray(
                    fac_all[g * HG : (g + 1) * HG][None, :]
                ),
            }
        )

    return in_maps


def kernel(**inputs):
    in_maps = make_in_maps(inputs)
    nc = _get_program()
    res = run_bass_kernel_spmd(nc, in_maps, list(range(N_CORES)))

    out = np.empty((B, T, D), dtype=np.float32)
    q = T // 4
    for b in range(B):
        even = res.results[2 * b]["out"]
        odd = res.results[2 * b + 1]["out"]
        out[b, 0:q] = even[0:q]
        out[b, q : 2 * q] = odd[0:q]
        out[b, 2 * q : 3 * q] = even[q : 2 * q]
        out[b, 3 * q : 4 * q] = odd[q : 2 * q]
    return out


if __name__ == "__main__":
    rng = np.random.default_rng(0)
    print("building program...")
    _get_program()
    print("built ok")

